# revision 1
# baseline (speedup 1.0000x reference)
"""2-layer GCN encoder on 8 TRN2 NeuronCores (Bass/Tile).

Sharding: node (dst) sharding. Each core owns SLOTS windows of 128 node
slots. The host groups nodes into windows by (in-degree-from-lower-half,
in-degree-from-upper-half) so windows are degree-homogeneous, then pairs
windows of similar max-degree across the 8 cores so one compiled program
(identical loop bounds) serves every core with minimal padding.

Layer math (projection pulled through the segment-sum; exact up to fp
reassociation):
    h   = relu(segsum((x @ W1_rel.T)[src]) + x @ W1_root.T + b1)
    out =      segsum((h @ W2_rel.T)[src]) + h @ W2_root.T + b2

Per core: compute the projected table rows for its own nodes, AllGather
the full table, dma_gather the rows for its in-edges (int16 indices; the
table is addressed in two halves so indices fit int16), and accumulate
the segment-sum in PSUM via identity matmuls: the gather list is
degree-slotted so the message for node-slot p always lands on SBUF
partition p.
"""

import sys

sys.path.insert(0, "/opt/trn_rl_repo")

import numpy as np

import concourse.bacc as bacc
import concourse.bass as bass
import concourse.mybir as mybir
import concourse.tile as tile
from concourse.bass_utils import run_bass_kernel_spmd
from concourse.masks import make_identity

P = 128
NCORES = 8
NB = 4  # edge-chunks accumulated per matmul (wide-PSUM lanes)

DEFAULT_CFG = dict(
    N=50000,   # real nodes
    F=96,      # input features
    H=128,     # hidden
    O=64,      # output features
    SLOTS=49,  # windows per core (NCORES*SLOTS*128 >= N, and N/2 <= NCORES*SLOTS*64)
    GMAX_COLS=16,  # max gather-group width in columns (128 idxs each)
)


def _derived(cfg):
    slots = cfg["SLOTS"]
    npc = slots * P              # node slots per core
    ntot = NCORES * npc          # total node slots
    half = ntot // 2             # table-half boundary (slot space)
    nhalf = cfg["N"] // 2        # real nodes per half (by original id)
    wph = half // P              # windows per half == 4*SLOTS
    assert wph == 4 * slots
    assert nhalf <= half - 1, "need at least one pad slot per half"
    assert half - 1 < 2**15, "table half must fit int16 indexing"
    return npc, ntot, half, nhalf, wph


def _group_windows(wKA, wKB):
    """Group windows into quads of similar (KA, KB) to minimize the
    per-quad (max KA + max KB) padding, then order quads by their maxes so
    rank-i quads from the two halves pair up with similar bounds."""
    nw = len(wKA)
    order = list(np.argsort(-(wKA.astype(np.int64) + wKB)))
    remset = set(order)
    quads = []
    for _ in range(nw // 4):
        seed = next(i for i in order if i in remset)
        remset.discard(seed)
        cands = [i for i in order if i in remset]
        cands.sort(
            key=lambda i: abs(int(wKA[i]) - int(wKA[seed]))
            + abs(int(wKB[i]) - int(wKB[seed]))
        )
        picks = cands[:3]
        for p in picks:
            remset.discard(p)
        quads.append([seed] + picks)
    quads.sort(key=lambda q: (max(int(wKA[i]) for i in q),
                              max(int(wKB[i]) for i in q)))
    return quads


def _make_plan(src, dst, cfg):
    """Host-side planning. src/dst int32 arrays, self-loops removed."""
    N = cfg["N"]
    slots = cfg["SLOTS"]
    npc, ntot, half, nhalf, wph = _derived(cfg)

    is_a = src < nhalf
    degA = np.bincount(dst[is_a], minlength=N).astype(np.int64)
    degB = np.bincount(dst[~is_a], minlength=N).astype(np.int64)

    node_dev = np.full(N, -1, np.int32)
    node_slot = np.full(N, -1, np.int32)
    node_part = np.full(N, -1, np.int32)
    node_of = np.full((NCORES, slots, P), -1, np.int64)
    KA = np.zeros(slots, np.int64)
    KB = np.zeros(slots, np.int64)
    pad_pos = [None, None]  # one pad slot position per table half

    for hf in (0, 1):
        nodes = np.arange(hf * nhalf, (hf + 1) * nhalf)
        # snake order: within each degA stratum alternate degB direction so
        # stratum-boundary windows stay degB-homogeneous
        sec = np.where(degA[nodes] % 2 == 1, -degB[nodes], degB[nodes])
        order = np.lexsort((sec, degA[nodes]))
        slot_list = np.concatenate(
            [nodes[order], np.full(half - nhalf, -1, np.int64)]
        )
        windows = slot_list.reshape(wph, P)
        wmask = windows >= 0
        wKA = np.where(wmask, degA[np.maximum(windows, 0)], 0).max(axis=1)
        wKB = np.where(wmask, degB[np.maximum(windows, 0)], 0).max(axis=1)
        groups4 = _group_windows(wKA, wKB)
        for i in range(slots):
            grp = groups4[i]
            KA[i] = max(KA[i], wKA[grp].max())
            KB[i] = max(KB[i], wKB[grp].max())
            for j, w in enumerate(grp):
                d = hf * 4 + (i + j) % 4
                members = windows[w]
                node_of[d, i] = members
                real = members >= 0
                parts = np.nonzero(real)[0]
                node_dev[members[real]] = d
                node_slot[members[real]] = i
                node_part[members[real]] = parts
                if pad_pos[hf] is None and (~real).any():
                    p0 = int(np.nonzero(~real)[0][0])
                    pad_pos[hf] = d * npc + i * P + p0
    assert pad_pos[0] is not None and pad_pos[1] is not None
    assert (node_dev >= 0).all()

    pos = node_dev.astype(np.int64) * npc + node_slot * P + node_part

    colbaseA = np.concatenate([[0], np.cumsum(KA)])
    colbaseB = np.concatenate([[0], np.cumsum(KB)])
    LA = int(colbaseA[-1]) * P
    LB = int(colbaseB[-1]) * P

    def edge_fill(sel, colbase, Ltot, pad_val, sub):
        flat = np.full((NCORES, max(Ltot, 16)), pad_val, np.int64)
        pd = pos[dst[sel]]
        pv = pos[src[sel]] - sub
        order = np.argsort(pd, kind="stable")
        pd = pd[order]
        pv = pv[order]
        starts = np.searchsorted(pd, pd, side="left")
        rank = np.arange(len(pd)) - starts
        dev = pd // npc
        slot = (pd % npc) // P
        part = pd % P
        fpos = (colbase[slot] + rank) * P + part
        flat[dev, fpos] = pv
        assert flat.min() >= 0 and flat.max() < half
        # wrap: element i -> [i % 16, i // 16], then replicate block to 128 rows
        wrapped = flat.reshape(NCORES, -1, 16).transpose(0, 2, 1)
        return np.tile(wrapped, (1, 8, 1)).astype(np.int16)

    idxA = edge_fill(is_a, colbaseA, LA, pad_pos[0], 0)
    idxB = edge_fill(~is_a, colbaseB, LB, pad_pos[1] - half, half)

    def make_groups(K, colbase):
        groups = []  # (c0, c1)
        slot2group = [None] * slots
        c0 = 0
        cols = 0
        gmax = cfg["GMAX_COLS"]
        for s in range(slots):
            if cols > 0 and cols + K[s] > gmax:
                groups.append((c0, c0 + cols))
                c0 += cols
                cols = 0
            slot2group[s] = (len(groups), cols)
            cols += int(K[s])
        if cols > 0:
            groups.append((c0, c0 + cols))
        return groups, slot2group

    groupsA, s2gA = make_groups(KA, colbaseA)
    groupsB, s2gB = make_groups(KB, colbaseB)

    meta = dict(
        cfg=dict(cfg),
        KA=[int(v) for v in KA],
        KB=[int(v) for v in KB],
        LA=max(LA, 16),
        LB=max(LB, 16),
        groupsA=groupsA,
        groupsB=groupsB,
        s2gA=s2gA,
        s2gB=s2gB,
    )
    return dict(
        meta=meta,
        node_dev=node_dev,
        node_slot=node_slot,
        node_part=node_part,
        node_of=node_of,
        idxA=idxA,
        idxB=idxB,
    )


def _make_in_maps(plan, cfg, x, W1_rel, b1, W1_root, W2_rel, b2, W2_root):
    F, H, O = cfg["F"], cfg["H"], cfg["O"]
    slots = cfg["SLOTS"]
    npc, _, _, _, _ = _derived(cfg)
    node_of = plan["node_of"]

    w1relT = np.zeros((F + 1, H), np.float32)
    w1relT[:F] = W1_rel.T
    w1rootT = np.zeros((F + 1, H), np.float32)
    w1rootT[:F] = W1_root.T
    w1rootT[F] = b1
    # root weights replicated NB x at weight/NB: the root matmul then writes
    # (and start=True-initializes) the full wide PSUM tile, and the per-window
    # fold over the NB regions reconstitutes the exact root term
    w1rootT_w = np.tile(w1rootT / NB, (1, NB))
    w2relT = np.ascontiguousarray(W2_rel.T, dtype=np.float32)
    w2rootT = np.ascontiguousarray(W2_root.T, dtype=np.float32)
    w2rootT_w = np.tile(w2rootT / NB, (1, NB))
    b2bc = np.ascontiguousarray(np.broadcast_to(b2, (P, O)), dtype=np.float32)

    in_maps = []
    for d in range(NCORES):
        members = node_of[d].reshape(-1)  # [npc]
        real = members >= 0
        xT = np.zeros((F + 1, npc), np.float32)
        xT[:F, real] = x[members[real]].T
        xT[F] = 1.0
        valid = np.zeros((P, slots), np.float32)
        valid[:, :] = real.reshape(slots, P).T
        in_maps.append(
            dict(
                xT=xT,
                w1relT=w1relT,
                w1rootT=w1rootT_w,
                w2relT=w2relT,
                w2rootT=w2rootT_w,
                b2bc=b2bc,
                valid=valid,
                idxA=np.ascontiguousarray(plan["idxA"][d]),
                idxB=np.ascontiguousarray(plan["idxB"][d]),
            )
        )
    return in_maps


def _build_nc(meta):
    cfg = meta["cfg"]
    F, H, O = cfg["F"], cfg["H"], cfg["O"]
    slots = cfg["SLOTS"]
    npc, ntot, half, _, _ = _derived(cfg)
    KA, KB = meta["KA"], meta["KB"]
    f32 = mybir.dt.float32
    i16 = mybir.dt.int16
    RG = [list(range(NCORES))]

    nc = bacc.Bacc(
        "TRN2",
        target_bir_lowering=False,
        debug=False,
        num_devices=NCORES,
        # 4 SWDGE queues: dma_gather desc-gen runs on the Q7 core pair
        # selected by queue_num, so round-robin queues parallelize it 4x
        num_swdge_queues=4,
    )
    xT_d = nc.dram_tensor("xT", [F + 1, npc], f32, kind="ExternalInput")
    w1r_d = nc.dram_tensor("w1relT", [F + 1, H], f32, kind="ExternalInput")
    w1o_d = nc.dram_tensor("w1rootT", [F + 1, NB * H], f32, kind="ExternalInput")
    w2r_d = nc.dram_tensor("w2relT", [H, O], f32, kind="ExternalInput")
    w2o_d = nc.dram_tensor("w2rootT", [H, NB * O], f32, kind="ExternalInput")
    b2_d = nc.dram_tensor("b2bc", [P, O], f32, kind="ExternalInput")
    vld_d = nc.dram_tensor("valid", [P, slots], f32, kind="ExternalInput")
    ixA_d = nc.dram_tensor("idxA", [P, meta["LA"] // 16], i16, kind="ExternalInput")
    ixB_d = nc.dram_tensor("idxB", [P, meta["LB"] // 16], i16, kind="ExternalInput")
    out_d = nc.dram_tensor("out", [npc, O], f32, kind="ExternalOutput")

    xr_loc = nc.dram_tensor("xr_loc", [npc, H], f32)
    hr_loc = nc.dram_tensor("hr_loc", [npc, O], f32)
    xr_full = nc.dram_tensor("xr_full", [ntot, H], f32, addr_space="Shared")
    hr_full = nc.dram_tensor("hr_full", [ntot, O], f32, addr_space="Shared")

    with tile.TileContext(nc) as tc:
        with (
            tc.tile_pool(name="const", bufs=1) as cp,
            tc.tile_pool(name="work", bufs=3) as wp,
            tc.tile_pool(name="gath", bufs=6) as gp,
            tc.tile_pool(name="psum", bufs=2, space="PSUM") as pp,
        ):
            def load_const(tag, dram, shape, dtype=f32):
                t = cp.tile(shape, dtype, tag=tag)
                nc.sync.dma_start(out=t[:], in_=dram[:])
                return t

            xt = load_const("xt", xT_d, [F + 1, npc])
            w1r = load_const("w1r", w1r_d, [F + 1, H])
            w1o = load_const("w1o", w1o_d, [F + 1, NB * H])
            w2r = load_const("w2r", w2r_d, [H, O])
            w2o = load_const("w2o", w2o_d, [H, NB * O])
            b2 = load_const("b2", b2_d, [P, O])
            vld = load_const("vld", vld_d, [P, slots])
            ixA = load_const("ixA", ixA_d, [P, meta["LA"] // 16], i16)
            ixB = load_const("ixB", ixB_d, [P, meta["LB"] // 16], i16)
            ident = cp.tile([P, P], f32, tag="ident")
            make_identity(nc, ident[:])
            hT = cp.tile([P, npc], f32, tag="hT")

            # ---- phase 1: xr table (this core's rows of x @ W1_rel.T)
            for s in range(slots):
                ps = pp.tile([P, H], f32, tag="ps_big")
                nc.tensor.matmul(
                    ps[:],
                    lhsT=xt[:, s * P : (s + 1) * P],
                    rhs=w1r[:],
                    start=True,
                    stop=True,
                )
                t = wp.tile([P, H], f32, tag="stage")
                nc.vector.tensor_copy(t[:], ps[:])
                nc.sync.dma_start(out=xr_loc[s * P : (s + 1) * P, :], in_=t[:])
            nc.gpsimd.collective_compute(
                "AllGather",
                mybir.AluOpType.bypass,
                replica_groups=RG,
                ins=[xr_loc[:]],
                outs=[xr_full[:]],
            )

            emitted = {}
            gq = [0]  # round-robin SWDGE queue for gathers

            def gtile(layer, stream, s):
                groups = meta["groupsA"] if stream == 0 else meta["groupsB"]
                s2g = meta["s2gA"] if stream == 0 else meta["s2gB"]
                gid, goff = s2g[s]
                key = (layer, stream, gid)
                if key not in emitted:
                    c0, c1 = groups[gid]
                    L = (c1 - c0) * P
                    esz = H if layer == 1 else O
                    table = xr_full if layer == 1 else hr_full
                    half_ap = table[:half, :] if stream == 0 else table[half:, :]
                    ix = ixA if stream == 0 else ixB
                    t = gp.tile([P, (c1 - c0) * esz], f32, tag=f"g{stream}")
                    nc.gpsimd.dma_gather(
                        out_ap=t[:].rearrange("p (c e) -> p c e", e=esz),
                        in_ap=half_ap,
                        idxs_ap=ix[:, c0 * 8 : c1 * 8],
                        num_idxs=L,
                        num_idxs_reg=L,
                        elem_size=esz,
                        # >~1024 idxs in one packet overflows the packet
                        # limit on HW (sim doesn't model it) — split packets
                        single_packet=(L <= 1024),
                        queue_num=gq[0],
                    )
                    gq[0] = (gq[0] + 1) % 4
                    emitted[key] = t
                return emitted[key], goff

            def accumulate(layer, s, lhs0, wroot, esz, fold_extra=None):
                """Wide-PSUM accumulation: root (replicated NB-wide) plus
                NB-chunk batched identity matmuls; returns the folded
                [P, esz] sum in an SBUF tile."""
                ps = pp.tile([P, NB * esz], f32, tag="ps_big")
                batches = []
                for stream in (0, 1):
                    Ks = (KA if stream == 0 else KB)[s]
                    if Ks == 0:
                        continue
                    t, goff = gtile(layer, stream, s)
                    for c0 in range(0, Ks, NB):
                        nb = min(NB, Ks - c0)
                        batches.append((t, goff + c0, nb))
                nc.tensor.matmul(
                    ps[:],
                    lhsT=lhs0,
                    rhs=wroot[:],
                    start=True,
                    stop=(not batches),
                )
                for i, (t, c0, nb) in enumerate(batches):
                    nc.tensor.matmul(
                        ps[:, : nb * esz],
                        lhsT=ident[:],
                        rhs=t[:, c0 * esz : (c0 + nb) * esz],
                        start=False,
                        stop=(i == len(batches) - 1),
                    )
                z = wp.tile([P, esz], f32, tag="stage")
                nc.vector.tensor_copy(z[:], ps[:, :esz])
                for j in range(1, NB):
                    nc.vector.tensor_tensor(
                        out=z[:],
                        in0=z[:],
                        in1=ps[:, j * esz : (j + 1) * esz],
                        op=mybir.AluOpType.add,
                    )
                return z

            # ---- layer 1
            for s in range(slots):
                z = accumulate(1, s, xt[:, s * P : (s + 1) * P], w1o, H)
                h = wp.tile([P, H], f32, tag="stage")
                # relu(z)*v == relu(z*v) for v in {0,1}: fold the pad-node
                # mask into the activation's per-partition scale
                nc.scalar.activation(
                    h[:],
                    z[:],
                    mybir.ActivationFunctionType.Relu,
                    scale=vld[:, s : s + 1],
                )
                pt = pp.tile([P, P], f32, tag="ps_tr")
                nc.tensor.transpose(pt[:], h[:], ident[:])
                nc.vector.tensor_copy(hT[:, s * P : (s + 1) * P], pt[:])
                pr = pp.tile([P, O], f32, tag="ps_sm")
                nc.tensor.matmul(
                    pr[:],
                    lhsT=hT[:, s * P : (s + 1) * P],
                    rhs=w2r[:],
                    start=True,
                    stop=True,
                )
                hrt = wp.tile([P, O], f32, tag="small")
                nc.vector.tensor_copy(hrt[:], pr[:])
                nc.sync.dma_start(out=hr_loc[s * P : (s + 1) * P, :], in_=hrt[:])
            nc.gpsimd.collective_compute(
                "AllGather",
                mybir.AluOpType.bypass,
                replica_groups=RG,
                ins=[hr_loc[:]],
                outs=[hr_full[:]],
            )

            # ---- layer 2
            for s in range(slots):
                z = accumulate(2, s, hT[:, s * P : (s + 1) * P], w2o, O)
                ot = wp.tile([P, O], f32, tag="small")
                nc.vector.tensor_tensor(
                    out=ot[:], in0=z[:], in1=b2[:], op=mybir.AluOpType.add
                )
                nc.sync.dma_start(out=out_d[s * P : (s + 1) * P, :], in_=ot[:])

    nc.compile()
    return nc


_NC_CACHE = {}


def _meta_key(meta):
    return repr(
        (
            meta["cfg"],
            meta["KA"],
            meta["KB"],
            meta["groupsA"],
            meta["groupsB"],
        )
    )


def _run(inputs, cfg=None, trace=False):
    cfg = dict(DEFAULT_CFG if cfg is None else cfg)
    x = np.ascontiguousarray(np.asarray(inputs["x"], np.float32))
    ei = np.asarray(inputs["edge_index"])
    src = ei[0].astype(np.int64)
    dst = ei[1].astype(np.int64)
    keep = src != dst
    src = src[keep].astype(np.int32)
    dst = dst[keep].astype(np.int32)

    plan = _make_plan(src, dst, cfg)
    key = _meta_key(plan["meta"])
    if key not in _NC_CACHE:
        _NC_CACHE[key] = _build_nc(plan["meta"])
    nc = _NC_CACHE[key]

    in_maps = _make_in_maps(
        plan,
        cfg,
        x,
        np.asarray(inputs["W1_rel"], np.float32),
        np.asarray(inputs["b1"], np.float32),
        np.asarray(inputs["W1_root"], np.float32),
        np.asarray(inputs["W2_rel"], np.float32),
        np.asarray(inputs["b2"], np.float32),
        np.asarray(inputs["W2_root"], np.float32),
    )
    res = run_bass_kernel_spmd(
        nc, in_maps, list(range(NCORES)), trace=trace
    )

    N, O = cfg["N"], cfg["O"]
    out = np.empty((N, O), np.float32)
    local = plan["node_slot"] * P + plan["node_part"]
    for d in range(NCORES):
        sel = plan["node_dev"] == d
        out[sel] = res.results[d]["out"][local[sel]]
    return out, res


def kernel(**inputs) -> np.ndarray:
    out, _ = _run(inputs)
    return out



# revision 3
# speedup vs baseline: 1.3111x; 1.3111x over previous
"""2-layer GCN encoder on 8 TRN2 NeuronCores (Bass/Tile).

Sharding: node (dst) sharding. Each core owns SLOTS windows of 128 node
slots. The host groups nodes into windows by (in-degree-from-lower-half,
in-degree-from-upper-half) so windows are degree-homogeneous, then pairs
windows of similar max-degree across the 8 cores so one compiled program
(identical loop bounds) serves every core with minimal padding.

v2 design (vs v1):
  - All gathered tables are bf16, 128 elems/row (256 B — the dma_gather
    minimum) — halves gather HBM traffic and makes the identity-matmul
    segment-sum stream at full bf16 PE rate.
  - Layer 1 gathers RAW x rows (host-prepared, replicated bf16 table)
    instead of projected rows: kills the xr projection phase AND the
    first AllGather entirely.  Projection happens per-window after
    aggregation: z = aggT.T @ W1relT + xT_win.T @ W1rootT(+b1).
  - Layer 2 gathers h rows (bf16): h = relu(z) cast to bf16, stored to
    DRAM per window, one bf16 AllGather, then same agg->project scheme.

Per core: dma_gather the rows for its in-edges (int16 indices; tables
addressed in two halves so indices fit int16), accumulate the
segment-sum in PSUM via identity matmuls: the gather list is
degree-slotted so the message for node-slot p always lands on SBUF
partition p.
"""

import sys

sys.path.insert(0, "/opt/trn_rl_repo")

import ml_dtypes
import numpy as np

import concourse.bacc as bacc
import concourse.bass as bass
import concourse.mybir as mybir
import concourse.tile as tile
from concourse.bass_utils import run_bass_kernel_spmd
from concourse.masks import make_identity

P = 128
NCORES = 8
NB = 4  # edge-chunks accumulated per matmul (wide-PSUM lanes)
BF16 = ml_dtypes.bfloat16

DEFAULT_CFG = dict(
    N=50000,   # real nodes
    F=96,      # input features
    H=128,     # hidden
    O=64,      # output features
    SLOTS=49,  # windows per core (NCORES*SLOTS*128 >= N, and N/2 <= NCORES*SLOTS*64)
    GMAX_COLS=16,  # max gather-group width in columns (128 idxs each)
    GBUFS=6,       # gather pool depth
)


def _derived(cfg):
    slots = cfg["SLOTS"]
    npc = slots * P              # node slots per core
    ntot = NCORES * npc          # total node slots
    half = ntot // 2             # table-half boundary (slot space)
    nhalf = cfg["N"] // 2        # real nodes per half (by original id)
    wph = half // P              # windows per half == 4*SLOTS
    assert wph == 4 * slots
    assert nhalf <= half - 1, "need at least one pad slot per half"
    assert half - 1 < 2**15, "table half must fit int16 indexing"
    return npc, ntot, half, nhalf, wph


def _group_windows(wKA, wKB):
    """Group windows into quads of similar (KA, KB) to minimize the
    per-quad (max KA + max KB) padding, then order quads by their maxes so
    rank-i quads from the two halves pair up with similar bounds."""
    nw = len(wKA)
    order = list(np.argsort(-(wKA.astype(np.int64) + wKB)))
    remset = set(order)
    quads = []
    for _ in range(nw // 4):
        seed = next(i for i in order if i in remset)
        remset.discard(seed)
        cands = [i for i in order if i in remset]
        cands.sort(
            key=lambda i: abs(int(wKA[i]) - int(wKA[seed]))
            + abs(int(wKB[i]) - int(wKB[seed]))
        )
        picks = cands[:3]
        for p in picks:
            remset.discard(p)
        quads.append([seed] + picks)
    quads.sort(key=lambda q: (max(int(wKA[i]) for i in q),
                              max(int(wKB[i]) for i in q)))
    return quads


def _make_plan(src, dst, cfg):
    """Host-side planning. src/dst int32 arrays, self-loops removed."""
    N = cfg["N"]
    slots = cfg["SLOTS"]
    npc, ntot, half, nhalf, wph = _derived(cfg)

    is_a = src < nhalf
    degA = np.bincount(dst[is_a], minlength=N).astype(np.int64)
    degB = np.bincount(dst[~is_a], minlength=N).astype(np.int64)

    node_dev = np.full(N, -1, np.int32)
    node_slot = np.full(N, -1, np.int32)
    node_part = np.full(N, -1, np.int32)
    node_of = np.full((NCORES, slots, P), -1, np.int64)
    KA = np.zeros(slots, np.int64)
    KB = np.zeros(slots, np.int64)
    pad_pos = [None, None]  # one pad slot position per table half

    for hf in (0, 1):
        nodes = np.arange(hf * nhalf, (hf + 1) * nhalf)
        # snake order: within each degA stratum alternate degB direction so
        # stratum-boundary windows stay degB-homogeneous
        sec = np.where(degA[nodes] % 2 == 1, -degB[nodes], degB[nodes])
        order = np.lexsort((sec, degA[nodes]))
        slot_list = np.concatenate(
            [nodes[order], np.full(half - nhalf, -1, np.int64)]
        )
        windows = slot_list.reshape(wph, P)
        wmask = windows >= 0
        wKA = np.where(wmask, degA[np.maximum(windows, 0)], 0).max(axis=1)
        wKB = np.where(wmask, degB[np.maximum(windows, 0)], 0).max(axis=1)
        groups4 = _group_windows(wKA, wKB)
        for i in range(slots):
            grp = groups4[i]
            KA[i] = max(KA[i], wKA[grp].max())
            KB[i] = max(KB[i], wKB[grp].max())
            for j, w in enumerate(grp):
                d = hf * 4 + (i + j) % 4
                members = windows[w]
                node_of[d, i] = members
                real = members >= 0
                parts = np.nonzero(real)[0]
                node_dev[members[real]] = d
                node_slot[members[real]] = i
                node_part[members[real]] = parts
                if pad_pos[hf] is None and (~real).any():
                    p0 = int(np.nonzero(~real)[0][0])
                    pad_pos[hf] = d * npc + i * P + p0
    assert pad_pos[0] is not None and pad_pos[1] is not None
    assert (node_dev >= 0).all()

    pos = node_dev.astype(np.int64) * npc + node_slot * P + node_part

    colbaseA = np.concatenate([[0], np.cumsum(KA)])
    colbaseB = np.concatenate([[0], np.cumsum(KB)])
    LA = int(colbaseA[-1]) * P
    LB = int(colbaseB[-1]) * P

    def edge_fill(sel, colbase, Ltot, pad_val, sub):
        flat = np.full((NCORES, max(Ltot, 16)), pad_val, np.int64)
        pd = pos[dst[sel]]
        pv = pos[src[sel]] - sub
        order = np.argsort(pd, kind="stable")
        pd = pd[order]
        pv = pv[order]
        starts = np.searchsorted(pd, pd, side="left")
        rank = np.arange(len(pd)) - starts
        dev = pd // npc
        slot = (pd % npc) // P
        part = pd % P
        fpos = (colbase[slot] + rank) * P + part
        flat[dev, fpos] = pv
        assert flat.min() >= 0 and flat.max() < half
        # wrap: element i -> [i % 16, i // 16], then replicate block to 128 rows
        wrapped = flat.reshape(NCORES, -1, 16).transpose(0, 2, 1)
        return np.tile(wrapped, (1, 8, 1)).astype(np.int16)

    idxA = edge_fill(is_a, colbaseA, LA, pad_pos[0], 0)
    idxB = edge_fill(~is_a, colbaseB, LB, pad_pos[1] - half, half)

    def make_groups(K, colbase):
        groups = []  # (c0, c1)
        slot2group = [None] * slots
        c0 = 0
        cols = 0
        gmax = cfg["GMAX_COLS"]
        for s in range(slots):
            if cols > 0 and cols + K[s] > gmax:
                groups.append((c0, c0 + cols))
                c0 += cols
                cols = 0
            slot2group[s] = (len(groups), cols)
            cols += int(K[s])
        if cols > 0:
            groups.append((c0, c0 + cols))
        return groups, slot2group

    groupsA, s2gA = make_groups(KA, colbaseA)
    groupsB, s2gB = make_groups(KB, colbaseB)

    meta = dict(
        cfg=dict(cfg),
        KA=[int(v) for v in KA],
        KB=[int(v) for v in KB],
        LA=max(LA, 16),
        LB=max(LB, 16),
        groupsA=groupsA,
        groupsB=groupsB,
        s2gA=s2gA,
        s2gB=s2gB,
    )
    return dict(
        meta=meta,
        node_dev=node_dev,
        node_slot=node_slot,
        node_part=node_part,
        node_of=node_of,
        idxA=idxA,
        idxB=idxB,
    )


def _make_in_maps(plan, cfg, x, W1_rel, b1, W1_root, W2_rel, b2, W2_root):
    F, H, O = cfg["F"], cfg["H"], cfg["O"]
    slots = cfg["SLOTS"]
    npc, ntot, _, _, _ = _derived(cfg)
    node_of = plan["node_of"]

    # x table in slot order, feature-padded to 128, bf16, replicated.
    x_tab = np.zeros((ntot, P), BF16)
    members_all = node_of.reshape(-1)  # [ntot]
    real_all = members_all >= 0
    x_tab[real_all, :F] = x[members_all[real_all]].astype(BF16)

    w1relT = np.zeros((P, H), np.float32)
    w1relT[:F] = W1_rel.T
    w1rootT = np.zeros((F + 1, H), np.float32)
    w1rootT[:F] = W1_root.T
    w1rootT[F] = b1
    w2relT = np.ascontiguousarray(W2_rel.T, dtype=np.float32)
    w2rootT = np.ascontiguousarray(W2_root.T, dtype=np.float32)
    b2bc = np.ascontiguousarray(np.broadcast_to(b2, (P, O)), dtype=np.float32)

    in_maps = []
    for d in range(NCORES):
        members = node_of[d].reshape(-1)  # [npc]
        real = members >= 0
        xT = np.zeros((F + 1, npc), BF16)
        xT[:F, real] = x[members[real]].T.astype(BF16)
        xT[F] = 1.0
        valid = np.zeros((P, slots), np.float32)
        valid[:, :] = real.reshape(slots, P).T
        in_maps.append(
            dict(
                x_tab=x_tab,
                xT=xT,
                w1relT=w1relT.astype(BF16),
                w1rootT=w1rootT.astype(BF16),
                w2relT=w2relT.astype(BF16),
                w2rootT=w2rootT.astype(BF16),
                b2bc=b2bc,
                valid=valid,
                idxA=np.ascontiguousarray(plan["idxA"][d]),
                idxB=np.ascontiguousarray(plan["idxB"][d]),
            )
        )
    return in_maps


def _build_nc(meta):
    cfg = meta["cfg"]
    F, H, O = cfg["F"], cfg["H"], cfg["O"]
    slots = cfg["SLOTS"]
    npc, ntot, half, _, _ = _derived(cfg)
    KA, KB = meta["KA"], meta["KB"]
    f32 = mybir.dt.float32
    bf16 = mybir.dt.bfloat16
    i16 = mybir.dt.int16
    RG = [list(range(NCORES))]

    nc = bacc.Bacc(
        "TRN2",
        target_bir_lowering=False,
        debug=False,
        num_devices=NCORES,
        # 4 SWDGE queues: dma_gather desc-gen runs on the Q7 core pair
        # selected by queue_num, so round-robin queues parallelize it 4x
        num_swdge_queues=4,
    )
    xtab_d = nc.dram_tensor("x_tab", [ntot, P], bf16, kind="ExternalInput")
    xT_d = nc.dram_tensor("xT", [F + 1, npc], bf16, kind="ExternalInput")
    w1r_d = nc.dram_tensor("w1relT", [P, H], bf16, kind="ExternalInput")
    w1o_d = nc.dram_tensor("w1rootT", [F + 1, H], bf16, kind="ExternalInput")
    w2r_d = nc.dram_tensor("w2relT", [H, O], bf16, kind="ExternalInput")
    w2o_d = nc.dram_tensor("w2rootT", [H, O], bf16, kind="ExternalInput")
    b2_d = nc.dram_tensor("b2bc", [P, O], f32, kind="ExternalInput")
    vld_d = nc.dram_tensor("valid", [P, slots], f32, kind="ExternalInput")
    ixA_d = nc.dram_tensor("idxA", [P, meta["LA"] // 16], i16, kind="ExternalInput")
    ixB_d = nc.dram_tensor("idxB", [P, meta["LB"] // 16], i16, kind="ExternalInput")
    out_d = nc.dram_tensor("out", [npc, O], f32, kind="ExternalOutput")

    h_loc = nc.dram_tensor("h_loc", [npc, H], bf16)
    h_full = nc.dram_tensor("h_full", [ntot, H], bf16, addr_space="Shared")

    with tile.TileContext(nc) as tc:
        with (
            tc.tile_pool(name="const", bufs=1) as cp,
            tc.tile_pool(name="work", bufs=3) as wp,
            tc.tile_pool(name="gath", bufs=cfg["GBUFS"]) as gp,
            tc.tile_pool(name="psum", bufs=2, space="PSUM") as pp,
        ):
            def load_const(tag, dram, shape, dtype=f32):
                t = cp.tile(shape, dtype, tag=tag)
                nc.sync.dma_start(out=t[:], in_=dram[:])
                return t

            xt = load_const("xt", xT_d, [F + 1, npc], bf16)
            w1r = load_const("w1r", w1r_d, [P, H], bf16)
            w1o = load_const("w1o", w1o_d, [F + 1, H], bf16)
            w2r = load_const("w2r", w2r_d, [H, O], bf16)
            w2o = load_const("w2o", w2o_d, [H, O], bf16)
            b2 = load_const("b2", b2_d, [P, O])
            vld = load_const("vld", vld_d, [P, slots])
            ixA = load_const("ixA", ixA_d, [P, meta["LA"] // 16], i16)
            ixB = load_const("ixB", ixB_d, [P, meta["LB"] // 16], i16)
            ident = cp.tile([P, P], bf16, tag="ident")
            make_identity(nc, ident[:])
            hT = cp.tile([P, npc], bf16, tag="hT")

            emitted = {}
            gq = [0]  # round-robin SWDGE queue for gathers

            def gtile(layer, stream, s):
                groups = meta["groupsA"] if stream == 0 else meta["groupsB"]
                s2g = meta["s2gA"] if stream == 0 else meta["s2gB"]
                gid, goff = s2g[s]
                key = (layer, stream, gid)
                if key not in emitted:
                    c0, c1 = groups[gid]
                    L = (c1 - c0) * P
                    table = xtab_d if layer == 1 else h_full
                    half_ap = table[:half, :] if stream == 0 else table[half:, :]
                    ix = ixA if stream == 0 else ixB
                    t = gp.tile([P, (c1 - c0) * P], bf16, tag=f"g{stream}")
                    nc.gpsimd.dma_gather(
                        out_ap=t[:].rearrange("p (c e) -> p c e", e=P),
                        in_ap=half_ap,
                        idxs_ap=ix[:, c0 * 8 : c1 * 8],
                        num_idxs=L,
                        num_idxs_reg=L,
                        elem_size=P,
                        # >~1024 idxs in one packet overflows the packet
                        # limit on HW (sim doesn't model it) — split packets
                        single_packet=(L <= 1024),
                        queue_num=gq[0],
                    )
                    gq[0] = (gq[0] + 1) % 4
                    emitted[key] = t
                return emitted[key], goff

            def aggregate(layer, s):
                """Wide-PSUM segment-sum of the gathered bf16 rows for
                window s; fold the lanes and return [P, P] bf16 agg rows
                in SBUF (node on partition), or None if the window has no
                in-edges."""
                batches = []
                for stream in (0, 1):
                    Ks = (KA if stream == 0 else KB)[s]
                    if Ks == 0:
                        continue
                    t, goff = gtile(layer, stream, s)
                    for c0 in range(0, Ks, NB):
                        nb = min(NB, Ks - c0)
                        batches.append((t, goff + c0, nb))
                if not batches:
                    return None
                batches.sort(key=lambda b: -b[2])
                nlanes = batches[0][2]
                ps = pp.tile([P, NB * P], f32, tag="ps_big")
                for i, (t, c0, nb) in enumerate(batches):
                    nc.tensor.matmul(
                        ps[:, : nb * P],
                        lhsT=ident[:],
                        rhs=t[:, c0 * P : (c0 + nb) * P],
                        start=(i == 0),
                        stop=(i == len(batches) - 1),
                    )
                # fold lanes; PSUM has one DVE read port, so each DVE op
                # may read at most one PSUM operand
                agg = wp.tile([P, P], bf16, tag="agg")
                if nlanes == 1:
                    nc.vector.tensor_copy(agg[:], ps[:, :P])
                else:
                    acc = wp.tile([P, P], f32, tag="acc")
                    nc.vector.tensor_copy(acc[:], ps[:, :P])
                    for j in range(1, nlanes):
                        nc.vector.tensor_tensor(
                            out=acc[:], in0=acc[:],
                            in1=ps[:, j * P : (j + 1) * P],
                            op=mybir.AluOpType.add,
                        )
                    nc.vector.tensor_copy(agg[:], acc[:])
                # transpose agg rows -> aggT (feature on partition)
                pt = pp.tile([P, P], bf16, tag="ps_tr")
                nc.tensor.transpose(pt[:], agg[:], ident[:])
                aggT = wp.tile([P, P], bf16, tag="aggT")
                nc.vector.tensor_copy(aggT[:], pt[:])
                return aggT

            # ---- layer 1
            for s in range(slots):
                aggT = aggregate(1, s)
                pz = pp.tile([P, H], f32, tag="ps_z")
                if aggT is not None:
                    nc.tensor.matmul(
                        pz[:], lhsT=aggT[:], rhs=w1r[:],
                        start=True, stop=False,
                    )
                nc.tensor.matmul(
                    pz[:],
                    lhsT=xt[:, s * P : (s + 1) * P],
                    rhs=w1o[:],
                    start=(aggT is None),
                    stop=True,
                )
                h = wp.tile([P, H], bf16, tag="h")
                # relu(z)*v == relu(z*v) for v in {0,1}: fold the pad-node
                # mask into the activation's per-partition scale
                nc.scalar.activation(
                    h[:],
                    pz[:],
                    mybir.ActivationFunctionType.Relu,
                    scale=vld[:, s : s + 1],
                )
                nc.sync.dma_start(out=h_loc[s * P : (s + 1) * P, :], in_=h[:])
                pt = pp.tile([P, P], bf16, tag="ps_tr")
                nc.tensor.transpose(pt[:], h[:], ident[:])
                nc.vector.tensor_copy(hT[:, s * P : (s + 1) * P], pt[:])
            nc.gpsimd.collective_compute(
                "AllGather",
                mybir.AluOpType.bypass,
                replica_groups=RG,
                ins=[h_loc[:]],
                outs=[h_full[:]],
            )

            # ---- layer 2
            for s in range(slots):
                aggT = aggregate(2, s)
                pz = pp.tile([P, O], f32, tag="ps_z2")
                if aggT is not None:
                    nc.tensor.matmul(
                        pz[:], lhsT=aggT[:], rhs=w2r[:],
                        start=True, stop=False,
                    )
                nc.tensor.matmul(
                    pz[:],
                    lhsT=hT[:, s * P : (s + 1) * P],
                    rhs=w2o[:],
                    start=(aggT is None),
                    stop=True,
                )
                ot = wp.tile([P, O], f32, tag="small")
                nc.vector.tensor_tensor(
                    out=ot[:], in0=pz[:], in1=b2[:], op=mybir.AluOpType.add
                )
                nc.sync.dma_start(out=out_d[s * P : (s + 1) * P, :], in_=ot[:])

    nc.compile()
    return nc


_NC_CACHE = {}


def _meta_key(meta):
    return repr(
        (
            meta["cfg"],
            meta["KA"],
            meta["KB"],
            meta["groupsA"],
            meta["groupsB"],
        )
    )


def _run(inputs, cfg=None, trace=False):
    cfg = dict(DEFAULT_CFG if cfg is None else cfg)
    x = np.ascontiguousarray(np.asarray(inputs["x"], np.float32))
    ei = np.asarray(inputs["edge_index"])
    src = ei[0].astype(np.int64)
    dst = ei[1].astype(np.int64)
    keep = src != dst
    src = src[keep].astype(np.int32)
    dst = dst[keep].astype(np.int32)

    plan = _make_plan(src, dst, cfg)
    key = _meta_key(plan["meta"])
    if key not in _NC_CACHE:
        _NC_CACHE[key] = _build_nc(plan["meta"])
    nc = _NC_CACHE[key]

    in_maps = _make_in_maps(
        plan,
        cfg,
        x,
        np.asarray(inputs["W1_rel"], np.float32),
        np.asarray(inputs["b1"], np.float32),
        np.asarray(inputs["W1_root"], np.float32),
        np.asarray(inputs["W2_rel"], np.float32),
        np.asarray(inputs["b2"], np.float32),
        np.asarray(inputs["W2_root"], np.float32),
    )
    res = run_bass_kernel_spmd(
        nc, in_maps, list(range(NCORES)), trace=trace
    )

    N, O = cfg["N"], cfg["O"]
    out = np.empty((N, O), np.float32)
    local = plan["node_slot"] * P + plan["node_part"]
    for d in range(NCORES):
        sel = plan["node_dev"] == d
        out[sel] = res.results[d]["out"][local[sel]]
    return out, res


def kernel(**inputs) -> np.ndarray:
    out, _ = _run(inputs)
    return out


# revision 6
# speedup vs baseline: 1.6506x; 1.2590x over previous
"""2-layer GCN encoder on 8 TRN2 NeuronCores (Bass/Tile).

Sharding: node (dst) sharding. Each core owns SLOTS windows of 128 node
slots. The host groups nodes into windows by (in-degree-from-lower-half,
in-degree-from-upper-half) so windows are degree-homogeneous, then packs
windows into slot-aligned groups of 8 (4 per half, local-search refined)
so one compiled program (identical loop bounds) serves every core with
minimal padding.

v3 design:
  - All gathered tables are bf16, 128 elems/row (256 B — the dma_gather
    minimum).  Layer 1 gathers RAW x rows (host-prepared, replicated
    bf16 table): no projection phase, no first AllGather.  Projection
    happens per-window after aggregation:
        z = aggT.T @ W1relT + xT_win.T @ W1rootT(+b1).
  - Layer 2 gathers h rows (bf16) after one bf16 AllGather.
  - Segment-sum via single-lane identity matmuls into PSUM (PE has
    headroom; DVE is kept off the critical path because GpSimd SWDGE
    descriptor generation contends with DVE for SBUF ports).
  - PSUM->SBUF moves and f32->bf16 casts run on the Scalar (ACT) engine.
  - Layer-2 gathers are PREPARE_ONLY on dedicated SWDGE queues (2,3):
    descriptor generation overlaps the layer-1 phase and the AllGather;
    the triggers (which carry the data dependency on the gathered table)
    fire after the collective lands.
"""

import sys

sys.path.insert(0, "/opt/trn_rl_repo")

import ml_dtypes
import numpy as np

import concourse.bacc as bacc
import concourse.bass as bass
import concourse.mybir as mybir
import concourse.tile as tile
from concourse.bass_utils import run_bass_kernel_spmd
from concourse.masks import make_identity

P = 128
NCORES = 8
BF16 = ml_dtypes.bfloat16

DEFAULT_CFG = dict(
    N=50000,   # real nodes
    F=96,      # input features
    H=128,     # hidden
    O=64,      # output features
    SLOTS=49,  # windows per core (NCORES*SLOTS*128 >= N, and N/2 <= NCORES*SLOTS*64)
    GMAX_COLS=24,  # max gather-group width in columns (128 idxs each)
    GBUFS=5,       # gather pool depth (per stream/layer tag)
    PREP_L2=False,  # prepare_only desc-gen for layer-2 gathers on queues 2/3
)


def _derived(cfg):
    slots = cfg["SLOTS"]
    npc = slots * P              # node slots per core
    ntot = NCORES * npc          # total node slots
    half = ntot // 2             # table-half boundary (slot space)
    nhalf = cfg["N"] // 2        # real nodes per half (by original id)
    wph = half // P              # windows per half == 4*SLOTS
    assert wph == 4 * slots
    assert nhalf <= half - 1, "need at least one pad slot per half"
    assert half - 1 < 2**15, "table half must fit int16 indexing"
    return npc, ntot, half, nhalf, wph


def _group_windows(wKA0, wKB0, wKA1, wKB1, slots):
    """Group each half's windows into quads of similar (KA, KB) (greedy
    seed + local-search refinement), then pair half-0 quads with half-1
    quads (Hungarian when scipy is available, rank pairing otherwise).
    Returns (quads0, quads1, KA, KB), slot-aligned."""

    def greedy(wKA, wKB):
        order = list(np.argsort(-(wKA.astype(np.int64) + wKB)))
        remset = set(order)
        quads = []
        for _ in range(slots):
            seed = next(i for i in order if i in remset)
            remset.discard(seed)
            cands = [i for i in order if i in remset]
            cands.sort(
                key=lambda i: abs(int(wKA[i]) - int(wKA[seed]))
                + abs(int(wKB[i]) - int(wKB[seed]))
            )
            picks = cands[:3]
            for p in picks:
                remset.discard(p)
            quads.append([seed] + picks)
        return quads

    def local_search(quads, wKA, wKB, iters=60000):
        rng = np.random.default_rng(0)
        quads = [list(q) for q in quads]

        def cost(q):
            return max(int(wKA[i]) for i in q) + max(int(wKB[i]) for i in q)

        nq = len(quads)
        pairs = rng.integers(0, nq, size=(iters, 2))
        mems = rng.integers(0, 4, size=(iters, 2))
        for (a, b), (ia, ib) in zip(pairs, mems):
            if a == b:
                continue
            qa, qb = quads[a], quads[b]
            old = cost(qa) + cost(qb)
            qa[ia], qb[ib] = qb[ib], qa[ia]
            if cost(qa) + cost(qb) > old:
                qa[ia], qb[ib] = qb[ib], qa[ia]
        return quads

    q0 = local_search(greedy(wKA0, wKB0), wKA0, wKB0)
    q1 = local_search(greedy(wKA1, wKB1), wKA1, wKB1)
    c0 = [(max(int(wKA0[i]) for i in q), max(int(wKB0[i]) for i in q)) for q in q0]
    c1 = [(max(int(wKA1[i]) for i in q), max(int(wKB1[i]) for i in q)) for q in q1]
    try:
        from scipy.optimize import linear_sum_assignment

        M = np.zeros((slots, slots))
        for i, (a0, b0) in enumerate(c0):
            for j, (a1, b1) in enumerate(c1):
                M[i, j] = max(a0, a1) + max(b0, b1)
        ri, ci = linear_sum_assignment(M)
        q0 = [q0[i] for i in ri]
        q1 = [q1[j] for j in ci]
        c0 = [c0[i] for i in ri]
        c1 = [c1[j] for j in ci]
    except ImportError:
        o0 = sorted(range(slots), key=lambda i: c0[i])
        o1 = sorted(range(slots), key=lambda j: c1[j])
        q0 = [q0[i] for i in o0]
        q1 = [q1[j] for j in o1]
        c0 = [c0[i] for i in o0]
        c1 = [c1[j] for j in o1]
    KA = np.array([max(a0, a1) for (a0, _), (a1, _) in zip(c0, c1)], np.int64)
    KB = np.array([max(b0, b1) for (_, b0), (_, b1) in zip(c0, c1)], np.int64)
    return q0, q1, KA, KB


def _make_plan(src, dst, cfg):
    """Host-side planning. src/dst int32 arrays, self-loops removed."""
    N = cfg["N"]
    slots = cfg["SLOTS"]
    npc, ntot, half, nhalf, wph = _derived(cfg)

    is_a = src < nhalf
    degA = np.bincount(dst[is_a], minlength=N).astype(np.int64)
    degB = np.bincount(dst[~is_a], minlength=N).astype(np.int64)

    node_dev = np.full(N, -1, np.int32)
    node_slot = np.full(N, -1, np.int32)
    node_part = np.full(N, -1, np.int32)
    node_of = np.full((NCORES, slots, P), -1, np.int64)
    pad_pos = [None, None]  # one pad slot position per table half

    windows_h = {}
    wK = {}
    for hf in (0, 1):
        nodes = np.arange(hf * nhalf, (hf + 1) * nhalf)
        # snake order: within each degA stratum alternate degB direction so
        # stratum-boundary windows stay degB-homogeneous
        sec = np.where(degA[nodes] % 2 == 1, -degB[nodes], degB[nodes])
        order = np.lexsort((sec, degA[nodes]))
        slot_list = np.concatenate(
            [nodes[order], np.full(half - nhalf, -1, np.int64)]
        )
        windows = slot_list.reshape(wph, P)
        wmask = windows >= 0
        windows_h[hf] = windows
        wK[hf] = (
            np.where(wmask, degA[np.maximum(windows, 0)], 0).max(axis=1),
            np.where(wmask, degB[np.maximum(windows, 0)], 0).max(axis=1),
        )

    quads0, quads1, KA, KB = _group_windows(
        wK[0][0], wK[0][1], wK[1][0], wK[1][1], slots
    )
    for hf, quads in ((0, quads0), (1, quads1)):
        windows = windows_h[hf]
        for i in range(slots):
            for j, w in enumerate(quads[i]):
                d = hf * 4 + (i + j) % 4
                members = windows[w]
                node_of[d, i] = members
                real = members >= 0
                parts = np.nonzero(real)[0]
                node_dev[members[real]] = d
                node_slot[members[real]] = i
                node_part[members[real]] = parts
                if pad_pos[hf] is None and (~real).any():
                    p0 = int(np.nonzero(~real)[0][0])
                    pad_pos[hf] = d * npc + i * P + p0
    assert pad_pos[0] is not None and pad_pos[1] is not None
    assert (node_dev >= 0).all()

    pos = node_dev.astype(np.int64) * npc + node_slot * P + node_part

    colbaseA = np.concatenate([[0], np.cumsum(KA)])
    colbaseB = np.concatenate([[0], np.cumsum(KB)])
    LA = int(colbaseA[-1]) * P
    LB = int(colbaseB[-1]) * P

    def edge_fill(sel, colbase, Ltot, pad_val, sub):
        flat = np.full((NCORES, max(Ltot, 16)), pad_val, np.int64)
        pd = pos[dst[sel]]
        pv = pos[src[sel]] - sub
        order = np.argsort(pd, kind="stable")
        pd = pd[order]
        pv = pv[order]
        starts = np.searchsorted(pd, pd, side="left")
        rank = np.arange(len(pd)) - starts
        dev = pd // npc
        slot = (pd % npc) // P
        part = pd % P
        fpos = (colbase[slot] + rank) * P + part
        flat[dev, fpos] = pv
        assert flat.min() >= 0 and flat.max() < half
        # wrap: element i -> [i % 16, i // 16], then replicate block to 128 rows
        wrapped = flat.reshape(NCORES, -1, 16).transpose(0, 2, 1)
        return np.tile(wrapped, (1, 8, 1)).astype(np.int16)

    idxA = edge_fill(is_a, colbaseA, LA, pad_pos[0], 0)
    idxB = edge_fill(~is_a, colbaseB, LB, pad_pos[1] - half, half)

    def make_groups(K, colbase):
        groups = []  # (c0, c1)
        slot2group = [None] * slots
        c0 = 0
        cols = 0
        gmax = cfg["GMAX_COLS"]
        for s in range(slots):
            if cols > 0 and cols + K[s] > gmax:
                groups.append((c0, c0 + cols))
                c0 += cols
                cols = 0
            slot2group[s] = (len(groups), cols)
            cols += int(K[s])
        if cols > 0:
            groups.append((c0, c0 + cols))
        return groups, slot2group

    groupsA, s2gA = make_groups(KA, colbaseA)
    groupsB, s2gB = make_groups(KB, colbaseB)

    meta = dict(
        cfg=dict(cfg),
        KA=[int(v) for v in KA],
        KB=[int(v) for v in KB],
        LA=max(LA, 16),
        LB=max(LB, 16),
        groupsA=groupsA,
        groupsB=groupsB,
        s2gA=s2gA,
        s2gB=s2gB,
    )
    return dict(
        meta=meta,
        node_dev=node_dev,
        node_slot=node_slot,
        node_part=node_part,
        node_of=node_of,
        idxA=idxA,
        idxB=idxB,
    )


def _make_in_maps(plan, cfg, x, W1_rel, b1, W1_root, W2_rel, b2, W2_root):
    F, H, O = cfg["F"], cfg["H"], cfg["O"]
    slots = cfg["SLOTS"]
    npc, ntot, _, _, _ = _derived(cfg)
    node_of = plan["node_of"]

    # x table in slot order, feature-padded to 128, bf16, replicated.
    x_tab = np.zeros((ntot, P), BF16)
    members_all = node_of.reshape(-1)  # [ntot]
    real_all = members_all >= 0
    x_tab[real_all, :F] = x[members_all[real_all]].astype(BF16)

    w1relT = np.zeros((P, H), np.float32)
    w1relT[:F] = W1_rel.T
    w1rootT = np.zeros((F + 1, H), np.float32)
    w1rootT[:F] = W1_root.T
    w1rootT[F] = b1
    w2relT = np.ascontiguousarray(W2_rel.T, dtype=np.float32)
    w2rootT = np.ascontiguousarray(W2_root.T, dtype=np.float32)
    b2bc = np.ascontiguousarray(np.broadcast_to(b2, (P, O)), dtype=np.float32)

    in_maps = []
    for d in range(NCORES):
        members = node_of[d].reshape(-1)  # [npc]
        real = members >= 0
        xT = np.zeros((F + 1, npc), BF16)
        xT[:F, real] = x[members[real]].T.astype(BF16)
        xT[F] = 1.0
        valid = np.zeros((P, slots), np.float32)
        valid[:, :] = real.reshape(slots, P).T
        in_maps.append(
            dict(
                x_tab=x_tab,
                xT=xT,
                w1relT=w1relT.astype(BF16),
                w1rootT=w1rootT.astype(BF16),
                w2relT=w2relT.astype(BF16),
                w2rootT=w2rootT.astype(BF16),
                b2bc=b2bc,
                valid=valid,
                idxA=np.ascontiguousarray(plan["idxA"][d]),
                idxB=np.ascontiguousarray(plan["idxB"][d]),
            )
        )
    return in_maps


def _build_nc(meta):
    cfg = meta["cfg"]
    F, H, O = cfg["F"], cfg["H"], cfg["O"]
    slots = cfg["SLOTS"]
    npc, ntot, half, _, _ = _derived(cfg)
    KA, KB = meta["KA"], meta["KB"]
    f32 = mybir.dt.float32
    bf16 = mybir.dt.bfloat16
    i16 = mybir.dt.int16
    RG = [list(range(NCORES))]
    prep_l2 = bool(cfg.get("PREP_L2", True))

    nc = bacc.Bacc(
        "TRN2",
        target_bir_lowering=False,
        debug=False,
        num_devices=NCORES,
        num_swdge_queues=4,
    )
    xtab_d = nc.dram_tensor("x_tab", [ntot, P], bf16, kind="ExternalInput")
    xT_d = nc.dram_tensor("xT", [F + 1, npc], bf16, kind="ExternalInput")
    w1r_d = nc.dram_tensor("w1relT", [P, H], bf16, kind="ExternalInput")
    w1o_d = nc.dram_tensor("w1rootT", [F + 1, H], bf16, kind="ExternalInput")
    w2r_d = nc.dram_tensor("w2relT", [H, O], bf16, kind="ExternalInput")
    w2o_d = nc.dram_tensor("w2rootT", [H, O], bf16, kind="ExternalInput")
    b2_d = nc.dram_tensor("b2bc", [P, O], f32, kind="ExternalInput")
    vld_d = nc.dram_tensor("valid", [P, slots], f32, kind="ExternalInput")
    ixA_d = nc.dram_tensor("idxA", [P, meta["LA"] // 16], i16, kind="ExternalInput")
    ixB_d = nc.dram_tensor("idxB", [P, meta["LB"] // 16], i16, kind="ExternalInput")
    out_d = nc.dram_tensor("out", [npc, O], f32, kind="ExternalOutput")

    h_loc = nc.dram_tensor("h_loc", [npc, H], bf16)
    h_full = nc.dram_tensor("h_full", [ntot, H], bf16, addr_space="Shared")

    with tile.TileContext(nc) as tc:
        with (
            tc.tile_pool(name="const", bufs=1) as cp,
            tc.tile_pool(name="work", bufs=3) as wp,
            tc.tile_pool(name="gath", bufs=cfg["GBUFS"]) as gp,
            tc.tile_pool(name="psum", bufs=2, space="PSUM") as pp,
        ):
            def load_const(tag, dram, shape, dtype=f32):
                t = cp.tile(shape, dtype, tag=tag)
                nc.sync.dma_start(out=t[:], in_=dram[:])
                return t

            xt = load_const("xt", xT_d, [F + 1, npc], bf16)
            w1r = load_const("w1r", w1r_d, [P, H], bf16)
            w1o = load_const("w1o", w1o_d, [F + 1, H], bf16)
            w2r = load_const("w2r", w2r_d, [H, O], bf16)
            w2o = load_const("w2o", w2o_d, [H, O], bf16)
            b2 = load_const("b2", b2_d, [P, O])
            vld = load_const("vld", vld_d, [P, slots])
            ixA = load_const("ixA", ixA_d, [P, meta["LA"] // 16], i16)
            ixB = load_const("ixB", ixB_d, [P, meta["LB"] // 16], i16)
            ident = cp.tile([P, P], bf16, tag="ident")
            make_identity(nc, ident[:])
            hT = cp.tile([P, npc], bf16, tag="hT")

            dma_sems = [nc.alloc_semaphore(f"swdge_dma_q{q}") for q in (2, 3)]
            emitted = {}
            gq = [0]  # round-robin SWDGE queue for gathers (all 4 queues)

            def gtile(layer, stream, s):
                groups = meta["groupsA"] if stream == 0 else meta["groupsB"]
                s2g = meta["s2gA"] if stream == 0 else meta["s2gB"]
                gid, goff = s2g[s]
                key = (layer, stream, gid)
                if key not in emitted:
                    c0, c1 = groups[gid]
                    L = (c1 - c0) * P
                    table = xtab_d if layer == 1 else h_full
                    half_ap = table[:half, :] if stream == 0 else table[half:, :]
                    ix = ixA if stream == 0 else ixB
                    t = gp.tile([P, (c1 - c0) * P], bf16, tag=f"g{stream}l{layer}")
                    kwargs = dict(
                        out_ap=t[:].rearrange("p (c e) -> p c e", e=P),
                        in_ap=half_ap,
                        idxs_ap=ix[:, c0 * 8 : c1 * 8],
                        num_idxs=L,
                        num_idxs_reg=L,
                        elem_size=P,
                        # >~1024 idxs in one packet overflows the packet
                        # limit on HW (sim doesn't model it) — split packets
                        single_packet=(L <= 1024),
                    )
                    if layer == 1 or not prep_l2:
                        q = gq1[0] if layer == 1 else gq2[0] + 2
                        nc.gpsimd.dma_gather(queue_num=q, **kwargs)
                    else:
                        # descriptor-gen early (no data dep); trigger carries
                        # the dependency on the gathered table
                        q = gq2[0] + 2
                        nc.gpsimd.dma_gather(
                            queue_num=q,
                            prepare_only=True,
                            sem=dma_sems[gq2[0]],
                            **kwargs,
                        )
                        nc.gpsimd.trigger_dma(count=None, queue_num=q)
                    if layer == 1:
                        gq1[0] = (gq1[0] + 1) % 2
                    else:
                        gq2[0] = (gq2[0] + 1) % 2
                    emitted[key] = t
                return emitted[key], goff

            def aggregate(layer, s):
                """Single-lane identity-matmul segment-sum of the gathered
                bf16 rows for window s, then transpose; returns the [P, P]
                bf16 aggT (feature on partition) in SBUF, or None if the
                window has no in-edges."""
                cols = []
                for stream in (0, 1):
                    Ks = (KA if stream == 0 else KB)[s]
                    if Ks == 0:
                        continue
                    t, goff = gtile(layer, stream, s)
                    cols.extend((t, goff + c) for c in range(Ks))
                if not cols:
                    return None
                ps1 = pp.tile([P, P], f32, tag="ps_acc")
                for i, (t, c) in enumerate(cols):
                    nc.tensor.matmul(
                        ps1[:],
                        lhsT=ident[:],
                        rhs=t[:, c * P : (c + 1) * P],
                        start=(i == 0),
                        stop=(i == len(cols) - 1),
                    )
                agg = wp.tile([P, P], bf16, tag="agg")
                nc.scalar.copy(agg[:], ps1[:])
                pt = pp.tile([P, P], bf16, tag="ps_tr")
                nc.tensor.transpose(pt[:], agg[:], ident[:])
                aggT = wp.tile([P, P], bf16, tag="aggT")
                nc.scalar.copy(aggT[:], pt[:])
                return aggT

            # ---- layer 1
            for s in range(slots):
                aggT = aggregate(1, s)
                pz = pp.tile([P, H], f32, tag="ps_z")
                if aggT is not None:
                    nc.tensor.matmul(
                        pz[:], lhsT=aggT[:], rhs=w1r[:],
                        start=True, stop=False,
                    )
                nc.tensor.matmul(
                    pz[:],
                    lhsT=xt[:, s * P : (s + 1) * P],
                    rhs=w1o[:],
                    start=(aggT is None),
                    stop=True,
                )
                h = wp.tile([P, H], bf16, tag="h")
                # relu(z)*v == relu(z*v) for v in {0,1}: fold the pad-node
                # mask into the activation's per-partition scale
                nc.scalar.activation(
                    h[:],
                    pz[:],
                    mybir.ActivationFunctionType.Relu,
                    scale=vld[:, s : s + 1],
                )
                nc.sync.dma_start(out=h_loc[s * P : (s + 1) * P, :], in_=h[:])
                pt = pp.tile([P, P], bf16, tag="ps_tr")
                nc.tensor.transpose(pt[:], h[:], ident[:])
                nc.scalar.copy(hT[:, s * P : (s + 1) * P], pt[:])
            nc.gpsimd.collective_compute(
                "AllGather",
                mybir.AluOpType.bypass,
                replica_groups=RG,
                ins=[h_loc[:]],
                outs=[h_full[:]],
            )

            # ---- layer 2
            for s in range(slots):
                aggT = aggregate(2, s)
                pz = pp.tile([P, O], f32, tag="ps_z2")
                if aggT is not None:
                    nc.tensor.matmul(
                        pz[:], lhsT=aggT[:], rhs=w2r[:],
                        start=True, stop=False,
                    )
                nc.tensor.matmul(
                    pz[:],
                    lhsT=hT[:, s * P : (s + 1) * P],
                    rhs=w2o[:],
                    start=(aggT is None),
                    stop=True,
                )
                ot = wp.tile([P, O], f32, tag="small")
                nc.vector.tensor_tensor(
                    out=ot[:], in0=pz[:], in1=b2[:], op=mybir.AluOpType.add
                )
                nc.sync.dma_start(out=out_d[s * P : (s + 1) * P, :], in_=ot[:])

    nc.compile()
    return nc


_NC_CACHE = {}


def _meta_key(meta):
    return repr(
        (
            meta["cfg"],
            meta["KA"],
            meta["KB"],
            meta["groupsA"],
            meta["groupsB"],
        )
    )


def _run(inputs, cfg=None, trace=False):
    cfg = dict(DEFAULT_CFG if cfg is None else cfg)
    x = np.ascontiguousarray(np.asarray(inputs["x"], np.float32))
    ei = np.asarray(inputs["edge_index"])
    src = ei[0].astype(np.int64)
    dst = ei[1].astype(np.int64)
    keep = src != dst
    src = src[keep].astype(np.int32)
    dst = dst[keep].astype(np.int32)

    plan = _make_plan(src, dst, cfg)
    key = _meta_key(plan["meta"])
    if key not in _NC_CACHE:
        _NC_CACHE[key] = _build_nc(plan["meta"])
    nc = _NC_CACHE[key]

    in_maps = _make_in_maps(
        plan,
        cfg,
        x,
        np.asarray(inputs["W1_rel"], np.float32),
        np.asarray(inputs["b1"], np.float32),
        np.asarray(inputs["W1_root"], np.float32),
        np.asarray(inputs["W2_rel"], np.float32),
        np.asarray(inputs["b2"], np.float32),
        np.asarray(inputs["W2_root"], np.float32),
    )
    res = run_bass_kernel_spmd(
        nc, in_maps, list(range(NCORES)), trace=trace
    )

    N, O = cfg["N"], cfg["O"]
    out = np.empty((N, O), np.float32)
    local = plan["node_slot"] * P + plan["node_part"]
    for d in range(NCORES):
        sel = plan["node_dev"] == d
        out[sel] = res.results[d]["out"][local[sel]]
    return out, res


def kernel(**inputs) -> np.ndarray:
    out, _ = _run(inputs)
    return out


# revision 10
# speedup vs baseline: 2.2683x; 1.3742x over previous
"""2-layer GCN encoder on 8 TRN2 NeuronCores (Bass/Tile).

Sharding: node (dst) sharding. Each core owns SLOTS windows of 128 node
slots. The host groups nodes into windows by (in-degree-from-lower-half,
in-degree-from-upper-half) so windows are degree-homogeneous, then packs
windows into slot-aligned groups of 8 (4 per half, local-search refined)
so one compiled program (identical loop bounds) serves every core with
minimal padding.

v3 design:
  - All gathered tables are bf16, 128 elems/row (256 B — the dma_gather
    minimum).  Layer 1 gathers RAW x rows (host-prepared, replicated
    bf16 table): no projection phase, no first AllGather.  Projection
    happens per-window after aggregation:
        z = aggT.T @ W1relT + xT_win.T @ W1rootT(+b1).
  - Layer 2 gathers h rows (bf16) after one bf16 AllGather.
  - Segment-sum via single-lane identity matmuls into PSUM (PE has
    headroom; DVE is kept off the critical path because GpSimd SWDGE
    descriptor generation contends with DVE for SBUF ports).
  - PSUM->SBUF moves and f32->bf16 casts run on the Scalar (ACT) engine.
  - Layer-2 gathers are PREPARE_ONLY on dedicated SWDGE queues (2,3):
    descriptor generation overlaps the layer-1 phase and the AllGather;
    the triggers (which carry the data dependency on the gathered table)
    fire after the collective lands.
"""

import sys

sys.path.insert(0, "/opt/trn_rl_repo")

import ml_dtypes
import numpy as np

import concourse.bacc as bacc
import concourse.bass as bass
import concourse.mybir as mybir
import concourse.tile as tile
from concourse.bass_utils import run_bass_kernel_spmd
from concourse.masks import make_identity

P = 128
NCORES = 8
BF16 = ml_dtypes.bfloat16

DEFAULT_CFG = dict(
    N=50000,   # real nodes
    F=96,      # input features
    H=128,     # hidden
    O=64,      # output features
    SLOTS=49,  # windows per core (NCORES*SLOTS*128 >= N, and N/2 <= NCORES*SLOTS*64)
    GMAX_COLS=8,  # gather-group width in columns (128 idxs each); 8 cols
                  # = 1024 idxs = the single-packet limit
    GBUFS=8,      # gather pool depth (per stream/layer tag)
)


def _derived(cfg):
    slots = cfg["SLOTS"]
    npc = slots * P              # node slots per core
    ntot = NCORES * npc          # total node slots
    half = ntot // 2             # table-half boundary (slot space)
    nhalf = cfg["N"] // 2        # real nodes per half (by original id)
    wph = half // P              # windows per half == 4*SLOTS
    assert wph == 4 * slots
    assert nhalf <= half - 1, "need at least one pad slot per half"
    assert half - 1 < 2**15, "table half must fit int16 indexing"
    return npc, ntot, half, nhalf, wph


def _group_windows(wKA0, wKB0, wKA1, wKB1, slots):
    """Group each half's windows into quads of similar (KA, KB) (greedy
    seed + local-search refinement), then pair half-0 quads with half-1
    quads (Hungarian when scipy is available, rank pairing otherwise).
    Returns (quads0, quads1, KA, KB), slot-aligned."""

    def greedy(wKA, wKB):
        order = list(np.argsort(-(wKA.astype(np.int64) + wKB)))
        remset = set(order)
        quads = []
        for _ in range(slots):
            seed = next(i for i in order if i in remset)
            remset.discard(seed)
            cands = [i for i in order if i in remset]
            cands.sort(
                key=lambda i: abs(int(wKA[i]) - int(wKA[seed]))
                + abs(int(wKB[i]) - int(wKB[seed]))
            )
            picks = cands[:3]
            for p in picks:
                remset.discard(p)
            quads.append([seed] + picks)
        return quads

    def local_search(quads, wKA, wKB, iters=60000):
        rng = np.random.default_rng(0)
        quads = [list(q) for q in quads]

        def cost(q):
            return max(int(wKA[i]) for i in q) + max(int(wKB[i]) for i in q)

        nq = len(quads)
        pairs = rng.integers(0, nq, size=(iters, 2))
        mems = rng.integers(0, 4, size=(iters, 2))
        for (a, b), (ia, ib) in zip(pairs, mems):
            if a == b:
                continue
            qa, qb = quads[a], quads[b]
            old = cost(qa) + cost(qb)
            qa[ia], qb[ib] = qb[ib], qa[ia]
            if cost(qa) + cost(qb) > old:
                qa[ia], qb[ib] = qb[ib], qa[ia]
        return quads

    q0 = local_search(greedy(wKA0, wKB0), wKA0, wKB0)
    q1 = local_search(greedy(wKA1, wKB1), wKA1, wKB1)
    c0 = [(max(int(wKA0[i]) for i in q), max(int(wKB0[i]) for i in q)) for q in q0]
    c1 = [(max(int(wKA1[i]) for i in q), max(int(wKB1[i]) for i in q)) for q in q1]
    try:
        from scipy.optimize import linear_sum_assignment

        M = np.zeros((slots, slots))
        for i, (a0, b0) in enumerate(c0):
            for j, (a1, b1) in enumerate(c1):
                M[i, j] = max(a0, a1) + max(b0, b1)
        ri, ci = linear_sum_assignment(M)
        q0 = [q0[i] for i in ri]
        q1 = [q1[j] for j in ci]
        c0 = [c0[i] for i in ri]
        c1 = [c1[j] for j in ci]
    except ImportError:
        o0 = sorted(range(slots), key=lambda i: c0[i])
        o1 = sorted(range(slots), key=lambda j: c1[j])
        q0 = [q0[i] for i in o0]
        q1 = [q1[j] for j in o1]
        c0 = [c0[i] for i in o0]
        c1 = [c1[j] for j in o1]
    KA = np.array([max(a0, a1) for (a0, _), (a1, _) in zip(c0, c1)], np.int64)
    KB = np.array([max(b0, b1) for (_, b0), (_, b1) in zip(c0, c1)], np.int64)
    return q0, q1, KA, KB


def _make_plan(src, dst, cfg):
    """Host-side planning. src/dst int32 arrays, self-loops removed."""
    N = cfg["N"]
    slots = cfg["SLOTS"]
    npc, ntot, half, nhalf, wph = _derived(cfg)

    is_a = src < nhalf
    degA = np.bincount(dst[is_a], minlength=N).astype(np.int64)
    degB = np.bincount(dst[~is_a], minlength=N).astype(np.int64)

    node_dev = np.full(N, -1, np.int32)
    node_slot = np.full(N, -1, np.int32)
    node_part = np.full(N, -1, np.int32)
    node_of = np.full((NCORES, slots, P), -1, np.int64)
    pad_pos = [None, None]  # one pad slot position per table half

    windows_h = {}
    wK = {}
    for hf in (0, 1):
        nodes = np.arange(hf * nhalf, (hf + 1) * nhalf)
        # snake order: within each degA stratum alternate degB direction so
        # stratum-boundary windows stay degB-homogeneous
        sec = np.where(degA[nodes] % 2 == 1, -degB[nodes], degB[nodes])
        order = np.lexsort((sec, degA[nodes]))
        slot_list = np.concatenate(
            [nodes[order], np.full(half - nhalf, -1, np.int64)]
        )
        windows = slot_list.reshape(wph, P)
        wmask = windows >= 0
        windows_h[hf] = windows
        wK[hf] = (
            np.where(wmask, degA[np.maximum(windows, 0)], 0).max(axis=1),
            np.where(wmask, degB[np.maximum(windows, 0)], 0).max(axis=1),
        )

    quads0, quads1, KA, KB = _group_windows(
        wK[0][0], wK[0][1], wK[1][0], wK[1][1], slots
    )
    for hf, quads in ((0, quads0), (1, quads1)):
        windows = windows_h[hf]
        for i in range(slots):
            for j, w in enumerate(quads[i]):
                d = hf * 4 + (i + j) % 4
                members = windows[w]
                node_of[d, i] = members
                real = members >= 0
                parts = np.nonzero(real)[0]
                node_dev[members[real]] = d
                node_slot[members[real]] = i
                node_part[members[real]] = parts
                if pad_pos[hf] is None and (~real).any():
                    p0 = int(np.nonzero(~real)[0][0])
                    pad_pos[hf] = d * npc + i * P + p0
    assert pad_pos[0] is not None and pad_pos[1] is not None
    assert (node_dev >= 0).all()

    pos = node_dev.astype(np.int64) * npc + node_slot * P + node_part

    colbaseA = np.concatenate([[0], np.cumsum(KA)])
    colbaseB = np.concatenate([[0], np.cumsum(KB)])
    LA = int(colbaseA[-1]) * P
    LB = int(colbaseB[-1]) * P

    def edge_fill(sel, colbase, Ltot, pad_val, sub):
        flat = np.full((NCORES, max(Ltot, 16)), pad_val, np.int64)
        pd = pos[dst[sel]]
        pv = pos[src[sel]] - sub
        order = np.argsort(pd, kind="stable")
        pd = pd[order]
        pv = pv[order]
        starts = np.searchsorted(pd, pd, side="left")
        rank = np.arange(len(pd)) - starts
        dev = pd // npc
        slot = (pd % npc) // P
        part = pd % P
        fpos = (colbase[slot] + rank) * P + part
        flat[dev, fpos] = pv
        assert flat.min() >= 0 and flat.max() < half
        # wrap: element i -> [i % 16, i // 16], then replicate block to 128 rows
        wrapped = flat.reshape(NCORES, -1, 16).transpose(0, 2, 1)
        return np.tile(wrapped, (1, 8, 1)).astype(np.int16)

    idxA = edge_fill(is_a, colbaseA, LA, pad_pos[0], 0)
    idxB = edge_fill(~is_a, colbaseB, LB, pad_pos[1] - half, half)

    def make_groups(colbase):
        # uniform column-granular groups of GMAX_COLS columns; a slot's
        # columns may span adjacent groups
        gmax = cfg["GMAX_COLS"]
        total = int(colbase[-1])
        groups = []
        c0 = 0
        while c0 < total:
            groups.append((c0, min(c0 + gmax, total)))
            c0 += gmax
        return groups

    groupsA = make_groups(colbaseA)
    groupsB = make_groups(colbaseB)

    meta = dict(
        cfg=dict(cfg),
        KA=[int(v) for v in KA],
        KB=[int(v) for v in KB],
        colbaseA=[int(v) for v in colbaseA],
        colbaseB=[int(v) for v in colbaseB],
        LA=max(LA, 16),
        LB=max(LB, 16),
        groupsA=groupsA,
        groupsB=groupsB,
    )
    return dict(
        meta=meta,
        node_dev=node_dev,
        node_slot=node_slot,
        node_part=node_part,
        node_of=node_of,
        idxA=idxA,
        idxB=idxB,
    )


def _make_in_maps(plan, cfg, x, W1_rel, b1, W1_root, W2_rel, b2, W2_root):
    F, H, O = cfg["F"], cfg["H"], cfg["O"]
    slots = cfg["SLOTS"]
    npc, ntot, _, _, _ = _derived(cfg)
    node_of = plan["node_of"]

    # x table in slot order, feature-padded to 128, bf16, replicated.
    x_tab = np.zeros((ntot, P), BF16)
    members_all = node_of.reshape(-1)  # [ntot]
    real_all = members_all >= 0
    x_tab[real_all, :F] = x[members_all[real_all]].astype(BF16)

    w1relT = np.zeros((P, H), np.float32)
    w1relT[:F] = W1_rel.T
    w1rootT = np.zeros((F + 1, H), np.float32)
    w1rootT[:F] = W1_root.T
    w1rootT[F] = b1
    w2relT = np.ascontiguousarray(W2_rel.T, dtype=np.float32)
    w2rootT = np.ascontiguousarray(W2_root.T, dtype=np.float32)
    b2bc = np.ascontiguousarray(np.broadcast_to(b2, (P, O)), dtype=np.float32)

    in_maps = []
    for d in range(NCORES):
        members = node_of[d].reshape(-1)  # [npc]
        real = members >= 0
        xT = np.zeros((F + 1, npc), BF16)
        xT[:F, real] = x[members[real]].T.astype(BF16)
        xT[F] = 1.0
        valid = np.zeros((P, slots), np.float32)
        valid[:, :] = real.reshape(slots, P).T
        in_maps.append(
            dict(
                x_tab=x_tab,
                xT=xT,
                w1relT=w1relT.astype(BF16),
                w1rootT=w1rootT.astype(BF16),
                w2relT=w2relT.astype(BF16),
                w2rootT=w2rootT.astype(BF16),
                b2bc=b2bc,
                valid=valid,
                idxA=np.ascontiguousarray(plan["idxA"][d]),
                idxB=np.ascontiguousarray(plan["idxB"][d]),
            )
        )
    return in_maps


def _build_nc(meta):
    cfg = meta["cfg"]
    F, H, O = cfg["F"], cfg["H"], cfg["O"]
    slots = cfg["SLOTS"]
    npc, ntot, half, _, _ = _derived(cfg)
    KA, KB = meta["KA"], meta["KB"]
    f32 = mybir.dt.float32
    bf16 = mybir.dt.bfloat16
    i16 = mybir.dt.int16
    RG = [list(range(NCORES))]

    nc = bacc.Bacc(
        "TRN2",
        target_bir_lowering=False,
        debug=False,
        num_devices=NCORES,
        num_swdge_queues=4,
    )
    xtab_d = nc.dram_tensor("x_tab", [ntot, P], bf16, kind="ExternalInput")
    xT_d = nc.dram_tensor("xT", [F + 1, npc], bf16, kind="ExternalInput")
    w1r_d = nc.dram_tensor("w1relT", [P, H], bf16, kind="ExternalInput")
    w1o_d = nc.dram_tensor("w1rootT", [F + 1, H], bf16, kind="ExternalInput")
    w2r_d = nc.dram_tensor("w2relT", [H, O], bf16, kind="ExternalInput")
    w2o_d = nc.dram_tensor("w2rootT", [H, O], bf16, kind="ExternalInput")
    b2_d = nc.dram_tensor("b2bc", [P, O], f32, kind="ExternalInput")
    vld_d = nc.dram_tensor("valid", [P, slots], f32, kind="ExternalInput")
    ixA_d = nc.dram_tensor("idxA", [P, meta["LA"] // 16], i16, kind="ExternalInput")
    ixB_d = nc.dram_tensor("idxB", [P, meta["LB"] // 16], i16, kind="ExternalInput")
    out_d = nc.dram_tensor("out", [npc, O], f32, kind="ExternalOutput")

    h_loc = nc.dram_tensor("h_loc", [npc, H], bf16)
    h_full = nc.dram_tensor("h_full", [ntot, H], bf16, addr_space="Shared")

    with tile.TileContext(nc) as tc:
        with (
            tc.tile_pool(name="const", bufs=1) as cp,
            tc.tile_pool(name="work", bufs=3) as wp,
            tc.tile_pool(name="gath", bufs=cfg["GBUFS"]) as gp,
            tc.tile_pool(name="psum", bufs=2, space="PSUM") as pp,
        ):
            def load_const(tag, dram, shape, dtype=f32):
                t = cp.tile(shape, dtype, tag=tag)
                nc.sync.dma_start(out=t[:], in_=dram[:])
                return t

            xt = load_const("xt", xT_d, [F + 1, npc], bf16)
            w1r = load_const("w1r", w1r_d, [P, H], bf16)
            w1o = load_const("w1o", w1o_d, [F + 1, H], bf16)
            w2r = load_const("w2r", w2r_d, [H, O], bf16)
            w2o = load_const("w2o", w2o_d, [H, O], bf16)
            b2 = load_const("b2", b2_d, [P, O])
            vld = load_const("vld", vld_d, [P, slots])
            ixA = load_const("ixA", ixA_d, [P, meta["LA"] // 16], i16)
            ixB = load_const("ixB", ixB_d, [P, meta["LB"] // 16], i16)
            ident = cp.tile([P, P], bf16, tag="ident")
            make_identity(nc, ident[:])
            hT = cp.tile([P, npc], bf16, tag="hT")

            emitted = {}
            gq = [0]  # round-robin SWDGE queue for gathers
            gmax = cfg["GMAX_COLS"]
            colbase = (meta["colbaseA"], meta["colbaseB"])
            groups = (meta["groupsA"], meta["groupsB"])

            def gtile(layer, stream, gid):
                key = (layer, stream, gid)
                if key not in emitted:
                    c0, c1 = groups[stream][gid]
                    L = (c1 - c0) * P
                    table = xtab_d if layer == 1 else h_full
                    half_ap = table[:half, :] if stream == 0 else table[half:, :]
                    ix = ixA if stream == 0 else ixB
                    t = gp.tile([P, (c1 - c0) * P], bf16, tag=f"g{stream}l{layer}")
                    nc.gpsimd.dma_gather(
                        out_ap=t[:].rearrange("p (c e) -> p c e", e=P),
                        in_ap=half_ap,
                        idxs_ap=ix[:, c0 * 8 : c1 * 8],
                        num_idxs=L,
                        num_idxs_reg=L,
                        elem_size=P,
                        # >~1024 idxs in one packet overflows the packet
                        # limit on HW (sim doesn't model it) — split packets
                        single_packet=(L <= 1024),
                        queue_num=gq[0],
                    )
                    gq[0] = (gq[0] + 1) % 4
                    emitted[key] = t
                return emitted[key]

            def aggregate(layer, s):
                """Single-lane identity-matmul segment-sum of the gathered
                bf16 rows for window s, then transpose; returns the [P, P]
                bf16 aggT (feature on partition) in SBUF, or None if the
                window has no in-edges."""
                cols = []
                for stream in (0, 1):
                    Ks = (KA if stream == 0 else KB)[s]
                    base = colbase[stream][s]
                    for c in range(base, base + Ks):
                        t = gtile(layer, stream, c // gmax)
                        cols.append((t, c % gmax))
                if not cols:
                    return None
                ps1 = pp.tile([P, P], f32, tag="ps_acc")
                for i, (t, c) in enumerate(cols):
                    nc.tensor.matmul(
                        ps1[:],
                        lhsT=ident[:],
                        rhs=t[:, c * P : (c + 1) * P],
                        start=(i == 0),
                        stop=(i == len(cols) - 1),
                    )
                agg = wp.tile([P, P], bf16, tag="agg")
                nc.scalar.copy(agg[:], ps1[:])
                pt = pp.tile([P, P], bf16, tag="ps_tr")
                nc.tensor.transpose(pt[:], agg[:], ident[:])
                aggT = wp.tile([P, P], bf16, tag="aggT")
                nc.scalar.copy(aggT[:], pt[:])
                return aggT

            # ---- layer 1
            for s in range(slots):
                aggT = aggregate(1, s)
                pz = pp.tile([P, H], f32, tag="ps_z")
                if aggT is not None:
                    nc.tensor.matmul(
                        pz[:], lhsT=aggT[:], rhs=w1r[:],
                        start=True, stop=False,
                    )
                nc.tensor.matmul(
                    pz[:],
                    lhsT=xt[:, s * P : (s + 1) * P],
                    rhs=w1o[:],
                    start=(aggT is None),
                    stop=True,
                )
                h = wp.tile([P, H], bf16, tag="h")
                # relu(z)*v == relu(z*v) for v in {0,1}: fold the pad-node
                # mask into the activation's per-partition scale
                nc.scalar.activation(
                    h[:],
                    pz[:],
                    mybir.ActivationFunctionType.Relu,
                    scale=vld[:, s : s + 1],
                )
                nc.sync.dma_start(out=h_loc[s * P : (s + 1) * P, :], in_=h[:])
                pt = pp.tile([P, P], bf16, tag="ps_tr")
                nc.tensor.transpose(pt[:], h[:], ident[:])
                nc.scalar.copy(hT[:, s * P : (s + 1) * P], pt[:])
            nc.gpsimd.collective_compute(
                "AllGather",
                mybir.AluOpType.bypass,
                replica_groups=RG,
                ins=[h_loc[:]],
                outs=[h_full[:]],
            )

            # ---- layer 2
            for s in range(slots):
                aggT = aggregate(2, s)
                pz = pp.tile([P, O], f32, tag="ps_z2")
                if aggT is not None:
                    nc.tensor.matmul(
                        pz[:], lhsT=aggT[:], rhs=w2r[:],
                        start=True, stop=False,
                    )
                nc.tensor.matmul(
                    pz[:],
                    lhsT=hT[:, s * P : (s + 1) * P],
                    rhs=w2o[:],
                    start=(aggT is None),
                    stop=True,
                )
                ot = wp.tile([P, O], f32, tag="small")
                nc.vector.tensor_tensor(
                    out=ot[:], in0=pz[:], in1=b2[:], op=mybir.AluOpType.add
                )
                nc.sync.dma_start(out=out_d[s * P : (s + 1) * P, :], in_=ot[:])

    nc.compile()
    return nc


_NC_CACHE = {}


def _meta_key(meta):
    return repr(
        (
            meta["cfg"],
            meta["KA"],
            meta["KB"],
            meta["groupsA"],
            meta["groupsB"],
        )
    )


def _run(inputs, cfg=None, trace=False):
    cfg = dict(DEFAULT_CFG if cfg is None else cfg)
    x = np.ascontiguousarray(np.asarray(inputs["x"], np.float32))
    ei = np.asarray(inputs["edge_index"])
    src = ei[0].astype(np.int64)
    dst = ei[1].astype(np.int64)
    keep = src != dst
    src = src[keep].astype(np.int32)
    dst = dst[keep].astype(np.int32)

    plan = _make_plan(src, dst, cfg)
    key = _meta_key(plan["meta"])
    if key not in _NC_CACHE:
        _NC_CACHE[key] = _build_nc(plan["meta"])
    nc = _NC_CACHE[key]

    in_maps = _make_in_maps(
        plan,
        cfg,
        x,
        np.asarray(inputs["W1_rel"], np.float32),
        np.asarray(inputs["b1"], np.float32),
        np.asarray(inputs["W1_root"], np.float32),
        np.asarray(inputs["W2_rel"], np.float32),
        np.asarray(inputs["b2"], np.float32),
        np.asarray(inputs["W2_root"], np.float32),
    )
    res = run_bass_kernel_spmd(
        nc, in_maps, list(range(NCORES)), trace=trace
    )

    N, O = cfg["N"], cfg["O"]
    out = np.empty((N, O), np.float32)
    local = plan["node_slot"] * P + plan["node_part"]
    for d in range(NCORES):
        sel = plan["node_dev"] == d
        out[sel] = res.results[d]["out"][local[sel]]
    return out, res


def kernel(**inputs) -> np.ndarray:
    out, _ = _run(inputs)
    return out
